# revision 5
# baseline (speedup 1.0000x reference)
"""ConnectionProductBlock on 8 TRN2 NeuronCores.

out[b, c*K + k, h, w] = am_out[b, c, h, w] * first_out[b, k, h, w]
  with B=16, C=8, K=64, H=W=56.

Strategy (data parallel over batch, 2 batches per core, no communication):
  - All device traffic is bf16 (rel err ~1e-2 max-elementwise, ~3e-3 l2,
    under the 2e-2 gate): halves the HBM-bound output traffic vs fp32.
    Host converts inputs fp32->bf16 and the returned bf16 output -> fp32.
  - SBUF layout: channels on partitions, hw (=3136) on the free dim so
    every DMA moves long contiguous runs (6.3KB per partition).
  - first_out for the core's 2 batches loads once as [128, 3136] bf16
    (partition = b*64 + k).
  - am needs a partition-broadcast (am[b, c] replicated across the 64 k
    partitions of batch b). Compute engines have fixed lane<->partition
    wiring, so the replication runs on the TensorEngine: a K=16 selector
    matmul sel_c.T @ am writes rep[p, f] = am[p//64, c, f] into PSUM
    (fp32) in 512-column (one bank) chunks.
  - PSUM fp32 operands cap DVE tensor_tensor at 1 elem/cycle, so Act and
    DVE first copy/convert rep into SBUF bf16 (split ~80/20 to balance
    engine load); the DVE then multiplies first2 * rep_sb -> out_t as
    all-bf16-SBUF tensor_tensor at 2 elem/cycle (2x_1p mode).
  - One 128-partition DMA per c ships out_t (both batches) to DRAM.
HBM traffic per core is ~7.3MB (6.4MB out + 0.9MB in), the bf16 minimum.
"""

import numpy as np

B, C, K, H, W = 16, 8, 64, 56, 56
HW = H * W  # 3136
NCORES = 8
BPC = B // NCORES  # batches per core = 2
MM = 512  # one PSUM bank of fp32 per matmul
# Copy groups per c: two 3-bank groups (3x512) + one 64-col tail. The fat
# groups amortize the Act engine's ~280ns fixed access cost per copy.
GROUPS = [(0, 1536), (1536, 1536), (3072, 64)]

_PROGRAMS = {}


def _build_program(
    repeat=1,
    do_compute=True,
    do_out_dma=True,
    dve_copy=((1, 0), (3, 0), (5, 0), (7, 0)),  # (c, group) copies on DVE + tails
    pool_mult=(2, 5),  # c's whose multiply runs on GPSIMD
    dual_ring=True,  # b=1 output DMAs on the gpsimd DGE ring
):
    """repeat>1 wraps the whole body in a hardware loop; bench-only.
    do_compute/do_out_dma isolate pipeline stages for benchmarking."""
    import contextlib

    import concourse.bacc as bacc
    import concourse.mybir as mybir
    import concourse.tile as tile

    nc = bacc.Bacc("TRN2", debug=False)
    # am data + per-c selector blocks on the free dim, one bf16 plane.
    # Partition = b*8 + c. One DMA covers data + selectors so each matmul
    # carries a single sem wait.
    amsel = nc.dram_tensor(
        "amsel", [BPC * C, HW + C * BPC * K], mybir.dt.bfloat16, kind="ExternalInput"
    )
    first = nc.dram_tensor(
        "first", [BPC * K, HW], mybir.dt.bfloat16, kind="ExternalInput"
    )
    out = nc.dram_tensor(
        "out", [BPC, C * K, HW], mybir.dt.bfloat16, kind="ExternalOutput"
    )

    with tile.TileContext(nc) as tc:
        with (
            tc.tile_pool(name="ins", bufs=1) as ins_pool,
            tc.tile_pool(name="rep", bufs=2, space="PSUM") as psum_pool,
            tc.tile_pool(name="rept", bufs=2, space="PSUM") as psumt_pool,
            tc.tile_pool(name="repsb", bufs=2) as repsb_pool,
            tc.tile_pool(name="outs", bufs=3) as out_pool,
            tc.For_i(0, repeat, 1) if repeat > 1 else contextlib.nullcontext(),
        ):
            # amsel loads first: the PE broadcast only needs amsel, so it
            # starts while first2 (4x bigger) is still in flight.
            am3 = ins_pool.tile([BPC * C, HW + C * BPC * K], mybir.dt.bfloat16)
            nc.sync.dma_start(out=am3[:], in_=amsel.ap())
            first2 = ins_pool.tile([BPC * K, HW], mybir.dt.bfloat16)
            nc.sync.dma_start(out=first2[:], in_=first.ap())

            out_ap = out.ap()
            for c in range(C):
                out_t = out_pool.tile([BPC * K, HW], mybir.dt.bfloat16, tag="out")
                if do_compute:
                    rep_sb = repsb_pool.tile(
                        [BPC * K, HW], mybir.dt.bfloat16, tag="repsb"
                    )
                    for gi, (g0, gn) in enumerate(GROUPS):
                        if gn == 64:
                            rep = psumt_pool.tile(
                                [BPC * K, 64], mybir.dt.float32, tag="rt", name="rep_t"
                            )
                        else:
                            rep = psum_pool.tile(
                                [BPC * K, 1536], mybir.dt.float32, tag="rep", name="rep"
                            )
                        for m0 in range(0, gn, MM):
                            mn = min(MM, gn - m0)
                            nc.tensor.matmul(
                                rep[:, m0 : m0 + mn],
                                lhsT=am3[
                                    :, HW + c * BPC * K : HW + (c + 1) * BPC * K
                                ],
                                rhs=am3[:, g0 + m0 : g0 + m0 + mn],
                                start=True,
                                stop=True,
                            )
                        # PSUM fp32 -> SBUF bf16 convert-copy (Act/DVE split)
                        if gn == 64 or (c, gi) in dve_copy:
                            nc.vector.tensor_copy(
                                rep_sb[:, g0 : g0 + gn], rep[:, 0:gn]
                            )
                        else:
                            nc.scalar.copy(rep_sb[:, g0 : g0 + gn], rep[:, 0:gn])
                    # all-bf16 SBUF tensor_tensor: 2x_1p on DVE; GPSIMD takes
                    # some c's to balance engine load.
                    eng = nc.gpsimd if c in pool_mult else nc.vector
                    eng.tensor_mul(out_t[:], first2[:], rep_sb[:])
                else:
                    nc.vector.memset(out_t[:, 0:2], 0.0)
                if do_out_dma:
                    # One DMA per batch ([64, HW] each, contiguous in DRAM),
                    # on separate DGE rings so both partition halves fly.
                    engs = (nc.sync, nc.gpsimd) if dual_ring else (nc.sync, nc.sync)
                    for b, eng in ((0, engs[0]), (1, engs[1])):
                        eng.dma_start(
                            out=out_ap[b, c * K : (c + 1) * K, :],
                            in_=out_t[b * K : (b + 1) * K, :],
                        )
    nc.compile()
    return nc


def _get_program(repeat=1, **variant):
    key = (repeat, tuple(sorted(variant.items())))
    if key not in _PROGRAMS:
        _PROGRAMS[key] = _build_program(repeat, **variant)
    return _PROGRAMS[key]


def _make_sel():
    # One [16, 128] selector block per c: sel[b*C + c, c*128 + b*64 + k] = 1
    sel = np.zeros((BPC * C, C * BPC * K), dtype=np.float32)
    for c in range(C):
        for b in range(BPC):
            sel[b * C + c, c * BPC * K + b * K : c * BPC * K + (b + 1) * K] = 1.0
    return sel


def _make_amsel(am_core):
    """am_core [BPC*C, HW] fp32 -> [BPC*C, HW + 1024] bf16 with the per-c
    selector blocks appended on the free dim."""
    import ml_dtypes

    bf16 = ml_dtypes.bfloat16
    return np.ascontiguousarray(
        np.concatenate([am_core.astype(bf16), _make_sel().astype(bf16)], axis=1)
    )


def _run(am_np, first_np, variant=None, **spmd_kwargs):
    import ml_dtypes

    from concourse.bass_utils import run_bass_kernel_spmd

    bf16 = ml_dtypes.bfloat16
    nc = _get_program(**(variant or {}))
    in_maps = []
    for i in range(NCORES):
        am_i = am_np[BPC * i : BPC * (i + 1)].reshape(BPC * C, HW)
        first_i = first_np[BPC * i : BPC * (i + 1)].reshape(BPC * K, HW)
        in_maps.append(
            {
                "amsel": _make_amsel(am_i),
                "first": np.ascontiguousarray(first_i.astype(bf16)),
            }
        )
    return run_bass_kernel_spmd(nc, in_maps, core_ids=list(range(NCORES)), **spmd_kwargs)


def kernel(am_out, first_out):
    am_np = np.asarray(am_out, dtype=np.float32).reshape(B, C, HW)
    first_np = np.asarray(first_out, dtype=np.float32).reshape(B, K, HW)
    res = _run(am_np, first_np)
    out = np.concatenate(
        [np.asarray(res.results[i]["out"], dtype=np.float32) for i in range(NCORES)],
        axis=0,
    )
    return out.reshape(B, C * K, H, W)
